# revision 1
# baseline (speedup 1.0000x reference)
"""BalanceLoss Trainium2 kernel.

Math restructuring (see reference _balance_loss):
  - pos_gt = (pos_sum >= B/2) and neg_gt are complementary, so maj/min
    groups partition the batch and their counts derive from pos_sum.
  - With y = (1-2t)*x (sign-folded logits, an input encoding choice):
      per-element BCE  = softplus(-x) + (1-t)*x = softplus(y)  exactly,
      easy <=> g < 1/3 <=> softplus(y) < tau,  tau = ln(1.5),
    so ONE softplus and ONE threshold serve both classes.
  - loss needs 5 per-column sums over the batch:
        pos = sum t          T   = sum v         Th  = sum_{hard} v
        S1  = sum_{t=1} v    S1h = sum_{t=1, hard} v      (v = softplus(y))
    then S0 = T - S1, S0h = Th - S1h, and per column:
        loss_c = maj_scale * Shard_maj + min_scale * S_min ;  total /(B*C)

Device (per core, data-parallel over batch), bf16 inputs y, t:
  ACT: q = exp(y) ; v = ln(q + 1) = softplus(y)      (one LUT table)
  DVE: msk = [v >= tau] ; a1 = t*v ; hv = msk*v ; h1 = t*hv
       (tensor_scalar hits the 4x DVE perf mode, tensor_tensor hits 2x;
        scalar_tensor_tensor / fused reduce ops measure 1x so are avoided)
  PE:  ones[128,1]^T @ {t, v, hv, a1, h1} -> 5 PSUM accumulators, emitted
       as one dependency-free burst per chunk so the PE clock stays ramped
Chunks ramp 512->2048->512 to shorten pipeline fill and drain.
Host: unshard + tiny per-column combine (scales, mean).
"""

import numpy as np

B_TOTAL = 131072
C = 128
N_CORES = 8
ROWS = B_TOTAL // N_CORES      # 16384 rows per core
FDMAX = 2048                   # largest free-dim chunk
MM_N = 512                     # matmul moving free dim (one PSUM bank)
N_STREAMS = 5
TAU = float(np.log(1.5))       # softplus(-ln2): easy/hard boundary

# chunk schedule: (j_rows_per_partition, view_index); free = 128*j
# covers rows [off, off+128*j) with off accumulated in order
CHUNK_J = [4, 4, 8, 16, 16, 16, 16, 16, 16, 8, 4, 4]   # sums to 128 j-rows
assert sum(CHUNK_J) * 128 == ROWS

_CACHE = {}


def _pin_act_tables():
    """Force the single LUT set containing both exp and ln so the kernel
    loads one ACT table instead of ping-ponging between two (1.3us/reload).
    Set indices must keep matching act_info.json, so empty the others."""
    import concourse.bacc as bacc
    import concourse.hw_specs as hw_specs

    if getattr(hw_specs, "_act_tables_pinned", False):
        return
    orig = hw_specs.get_activation_tables

    def patched(arch):
        tabs = dict(orig(arch))
        keep = "natural_log_exp_and_others"
        if keep in tabs:
            tabs = {n: (s if n == keep else set()) for n, s in tabs.items()}
        return tabs

    hw_specs._act_tables_pinned = True
    hw_specs.get_activation_tables = patched
    bacc.get_activation_tables = patched


def _build_nc():
    import concourse.bacc as bacc
    import concourse.tile as tile
    from concourse import mybir

    _pin_act_tables()

    f32 = mybir.dt.float32
    bf16 = mybir.dt.bfloat16
    AF = mybir.ActivationFunctionType
    OP = mybir.AluOpType

    nc = bacc.Bacc(None)
    yd = nc.dram_tensor("y", [ROWS, C], bf16, kind="ExternalInput")
    td = nc.dram_tensor("t", [ROWS, C], bf16, kind="ExternalInput")
    out = nc.dram_tensor("partials", [1, N_STREAMS * MM_N], f32,
                         kind="ExternalOutput")

    # chunk m covers rows [off, off+128*j): partition p holds rows
    # off+p*j .. off+(p+1)*j-1, contiguous (j c) in its free dim
    def view(d, off, j):
        return d[off : off + 128 * j].rearrange("(p j) c -> p (j c)", p=128)

    n_chunks = len(CHUNK_J)
    with tile.TileContext(nc) as tc:
        with (
            tc.tile_pool(name="singles", bufs=1) as singles,
            tc.tile_pool(name="io", bufs=4) as io,
            tc.tile_pool(name="work", bufs=4) as work,
            tc.tile_pool(name="psum", bufs=1, space="PSUM") as psum_pool,
        ):
            ones = singles.tile([128, 1], bf16)
            nc.vector.memset(ones, 1.0)
            acc = psum_pool.tile([1, N_STREAMS * MM_N], f32, tag="acc")
            # Warmup matmul consumes the ones-memset dependency so that
            # steady-state matmuls carry at most one sync wait (walrus
            # LDWEIGHTS codegen supports only one).
            warm = psum_pool.tile([1, 1], f32, tag="warm")
            nc.tensor.matmul(warm, ones, ones, start=True, stop=True)

            # first four chunks' loads are issued up front with y3 swapped
            # ahead of t2: ACT measurably stalls ~2.9us waiting for y3 on
            # the serial issue queue, while t2/t3 have ~3us of slack
            offs = [0]
            for j in CHUNK_J:
                offs.append(offs[-1] + 128 * j)
            pre_y, pre_t = {}, {}
            for k, nm in [(0, "y"), (0, "t"), (1, "y"), (1, "t"),
                          (2, "y"), (3, "y"), (2, "t"), (3, "t")]:
                fd = 128 * CHUNK_J[k]
                tile_ = io.tile([128, FDMAX], bf16, tag=nm, name=f"{nm}{k}")
                src = view(yd if nm == "y" else td, offs[k], CHUNK_J[k])
                nc.sync.dma_start(tile_[:, 0:fd], src)
                (pre_y if nm == "y" else pre_t)[k] = tile_

            for m, j in enumerate(CHUNK_J):
                fd = 128 * j
                if m < 4:
                    y = pre_y[m]
                    t = pre_t[m]
                else:
                    y = io.tile([128, FDMAX], bf16, tag="y")
                    t = io.tile([128, FDMAX], bf16, tag="t")
                    nc.sync.dma_start(y[:, 0:fd], view(yd, offs[m], j))
                    nc.sync.dma_start(t[:, 0:fd], view(td, offs[m], j))

                q = work.tile([128, FDMAX], bf16, tag="q")
                v = work.tile([128, FDMAX], bf16, tag="v")
                nc.scalar.activation(q[:, 0:fd], y[:, 0:fd], AF.Exp)
                nc.scalar.activation(v[:, 0:fd], q[:, 0:fd], AF.Ln, bias=1.0)

                msk = work.tile([128, FDMAX], bf16, tag="msk")
                hv = work.tile([128, FDMAX], bf16, tag="hv")
                a1 = work.tile([128, FDMAX], bf16, tag="a1")
                h1 = work.tile([128, FDMAX], bf16, tag="h1")

                # matmuls are emitted right after each stream's producer so
                # the PE starts each chunk's work as early as the in-order
                # queue allows (helps the pipeline-fill phase; identical in
                # steady state). Pool stays idle on purpose — any real Pool
                # load trips the power governor into 50%-duty throttling.
                first = m == 0
                last = m == n_chunks - 1

                def mm(s, mv):
                    for jj in range(fd // MM_N):
                        nc.tensor.matmul(
                            acc[:, s * MM_N : (s + 1) * MM_N],
                            ones[:, :],
                            mv[:, jj * MM_N : (jj + 1) * MM_N],
                            start=(first and jj == 0),
                            stop=(last and jj == fd // MM_N - 1),
                        )

                mm(0, t)
                mm(1, v)
                # msk first; a1 between msk and hv hides the DVE
                # write-to-read (RAW) stall on msk
                nc.vector.tensor_scalar(
                    msk[:, 0:fd], v[:, 0:fd], TAU, None, OP.is_ge)
                nc.vector.tensor_tensor(
                    a1[:, 0:fd], t[:, 0:fd], v[:, 0:fd], OP.mult)
                mm(3, a1)
                nc.vector.tensor_tensor(
                    hv[:, 0:fd], msk[:, 0:fd], v[:, 0:fd], OP.mult)
                mm(2, hv)
                nc.vector.tensor_tensor(
                    h1[:, 0:fd], t[:, 0:fd], hv[:, 0:fd], OP.mult)
                mm(4, h1)

            res = singles.tile([1, N_STREAMS * MM_N], f32)
            # split the PSUM->SBUF drain across DVE and ACT
            nc.vector.tensor_copy(res[:, 0 : 3 * MM_N], acc[:, 0 : 3 * MM_N])
            nc.scalar.copy(
                res[:, 3 * MM_N : 5 * MM_N], acc[:, 3 * MM_N : 5 * MM_N])
            nc.sync.dma_start(out[:, :], res)
    nc.finalize()
    return nc


def _get_nc():
    if "nc" not in _CACHE:
        _CACHE["nc"] = _build_nc()
    return _CACHE["nc"]


def _in_maps(pred, target):
    import ml_dtypes

    bf = ml_dtypes.bfloat16
    p32 = np.asarray(pred, dtype=np.float32)
    t32 = np.asarray(target, dtype=np.float32)
    y = ((1.0 - 2.0 * t32) * p32).astype(bf)   # exact sign flip of pred
    t = t32.astype(bf)
    return [
        {
            "y": np.ascontiguousarray(y[i * ROWS : (i + 1) * ROWS]),
            "t": np.ascontiguousarray(t[i * ROWS : (i + 1) * ROWS]),
        }
        for i in range(N_CORES)
    ]


def _combine(parts):
    """parts: [n_cores, 5, MM_N] raw psum rows -> final scalar loss."""
    # psum col q sums j-groups with (j % (MM_N//C)) == q//C at col q % C;
    # fold the leftover j-groups and cores.
    S = parts.reshape(-1, N_STREAMS, MM_N // C, C).sum(axis=(0, 2),
                                                       dtype=np.float64)
    pos, T, Th, s1, s1h = S
    B = float(B_TOTAL)
    s0, s0h = T - s1, Th - s1h
    bal = 0.5 * B
    pos_gt = pos >= bal
    maj_cnt = np.where(pos_gt, pos, B - pos)
    min_cnt = B - maj_cnt
    maj_scale = bal / np.maximum(maj_cnt, 1.0)
    min_scale = np.where(min_cnt > 0, (B - bal) / np.maximum(min_cnt, 1.0), 1.0)
    s_maj_hard = np.where(pos_gt, s1h, s0h)
    s_min = np.where(pos_gt, s0, s1)
    total = (maj_scale * s_maj_hard + min_scale * s_min).sum()
    return np.float32(total / (B * C))


def kernel(pred: np.ndarray, target: np.ndarray) -> np.ndarray:
    from concourse.bass_utils import run_bass_kernel_spmd

    nc = _get_nc()
    res = run_bass_kernel_spmd(
        nc, _in_maps(pred, target), core_ids=list(range(N_CORES)))
    parts = np.stack(
        [r["partials"].reshape(N_STREAMS, MM_N) for r in res.results])
    return _combine(parts)

